# revision 1
# baseline (speedup 1.0000x reference)
"""CrossAttention + residual + LayerNorm on 8 Trainium2 NeuronCores.

Reference computation (per batch b):
    q = x @ Wq + bq ; k = ctx @ Wk + bk ; v = ctx @ Wv + bv      (16 heads of 64)
    attn = softmax(q k^T / 8) ; out = attn @ v
    y = LayerNorm(out @ Wo + bo + x) * gamma + beta

Sharding: core c -> batch b = c//4, query rows [512*(c%4), 512*(c%4+1)).
Each core recomputes K/V projections for its batch (replication is ~82us of
PE time; an intra-chip collective for the alternative head-parallel split
would cost 250us+ at the measured ~30-60 GB/s collective bandwidth).

Layouts on core (SBUF, fp32):
    ctxT  [128, 8, 2048]   context^T   (feature f = 128*j + p)
    xT    [128, 8, 512]    x-slice^T
    qT    [128, 8, 512]    Q^T   feature-major; head h lives at rows 64*(h%2)
    kT_g  [128, 2, 2048]   K^T for the 4-head group g
    v_g   [128, 16, 4, 65] V natural per k-tile, per head-in-group, with a
                           ones column at [..., 64] (softmax denominator)
    attnT [128, 8, 512]    normalized attention output^T (feature-major)

Attention per head pair (A at partitions 0-63, B at 64-127, row-packed):
    S^T[k_tile, q] = (K^T tile).T @ Q^T        -> PSUM [128, 512]
    P^T = exp(S^T / 8)                         -> SBUF (ACT engine)
    O^T[65, 512] += (V_aug tile).T @ P^T       -> row 64 = sum_k P
    recip = 1 / O^T[64]; bcast via ones-matmul; attnT = O^T[0:64] * bcast

All matmuls run as float32r (1 cycle/row at free-dim>=256; fp32 would be 4).
"""

import numpy as np

import concourse.bacc as bacc
import concourse.bass as bass
import concourse.tile as tile
from concourse import mybir
from concourse.masks import make_identity

F32 = mybir.dt.float32
F32R = mybir.dt.float32r
AF = mybir.ActivationFunctionType

B = 2
N = 2048          # context length
D = 1024          # model dim
H = 16            # heads
HD = 64           # head dim
NQ = 512          # query rows per core
SCALE = HD ** -0.5
EPS = 1e-5
NG = 4            # head groups
GF = D // NG      # features per group (256)

_CACHE = {}


def _emit(nc):
    with nc.allow_low_precision(reason="fp32r matmul operands; rounding on write"):
        _emit_body(nc)


def _emit_body(nc):
    xs = nc.dram_tensor("xs", [NQ, D], F32, kind="ExternalInput")
    ctx = nc.dram_tensor("ctx", [N, D], F32, kind="ExternalInput")
    Wq = nc.dram_tensor("Wq", [D, D], F32, kind="ExternalInput")
    Wk = nc.dram_tensor("Wk", [D, D], F32, kind="ExternalInput")
    Wv = nc.dram_tensor("Wv", [D, D], F32, kind="ExternalInput")
    Wo = nc.dram_tensor("Wo", [D, D], F32, kind="ExternalInput")
    bq = nc.dram_tensor("bq", [D], F32, kind="ExternalInput")
    bk = nc.dram_tensor("bk", [D], F32, kind="ExternalInput")
    bv = nc.dram_tensor("bv", [D], F32, kind="ExternalInput")
    bo = nc.dram_tensor("bo", [D], F32, kind="ExternalInput")
    gamma = nc.dram_tensor("gamma", [D], F32, kind="ExternalInput")
    beta = nc.dram_tensor("beta", [D], F32, kind="ExternalInput")
    y = nc.dram_tensor("y", [NQ, D], F32, kind="ExternalOutput")

    def bcast_row(dram_vec):
        # [D] -> [128, D] DMA broadcast (partition step 0)
        a = dram_vec.ap()
        return bass.AP(tensor=a.tensor, offset=0, ap=[[0, 128]] + a.ap)

    def col_view(dram_vec):
        # [D] -> [128, 8] with [p, j] = vec[128*j + p]
        return dram_vec.ap().rearrange("(j p) -> p j", p=128)

    with tile.TileContext(nc) as tc, \
         tc.tile_pool(name="const", bufs=1) as const, \
         tc.tile_pool(name="resid", bufs=1) as resid, \
         tc.tile_pool(name="qTp", bufs=1) as qT_pool, \
         tc.tile_pool(name="attnTp", bufs=1) as attnT_pool:
        ident = const.tile([128, 128], F32)
        make_identity(nc, ident)
        ones64f = const.tile([1, 64], F32)
        nc.vector.memset(ones64f, 1.0)
        ones64 = const.tile([1, 64], F32R)
        nc.vector.tensor_copy(out=ones64, in_=ones64f)
        vones = const.tile([128, 16, 4, 1], F32)
        nc.vector.memset(vones, 1.0)
        eps_t = const.tile([128, 1], F32)
        nc.vector.memset(eps_t, EPS)
        bq_c = const.tile([128, 8], F32)
        nc.sync.dma_start(out=bq_c, in_=col_view(bq))
        bk_c = const.tile([128, 8], F32)
        nc.sync.dma_start(out=bk_c, in_=col_view(bk))
        bv_b = const.tile([128, D], F32)
        nc.sync.dma_start(out=bv_b, in_=bcast_row(bv))
        bo_b = const.tile([128, D], F32)
        nc.sync.dma_start(out=bo_b, in_=bcast_row(bo))
        gamma_b = const.tile([128, D], F32)
        nc.sync.dma_start(out=gamma_b, in_=bcast_row(gamma))
        beta_b = const.tile([128, D], F32)
        nc.sync.dma_start(out=beta_b, in_=bcast_row(beta))

        xbo = resid.tile([128, 4, D], F32)   # x-slice + bo, token t = 128*i + p
        nc.sync.dma_start(out=xbo, in_=xs.ap().rearrange("(i p) d -> p i d", p=128))
        for i in range(4):
            nc.vector.tensor_add(out=xbo[:, i, :], in0=xbo[:, i, :], in1=bo_b)

        qT = qT_pool.tile([128, 8, NQ], F32R)
        attnT = attnT_pool.tile([128, 8, NQ], F32R)

        with tc.tile_pool(name="ctxT", bufs=1) as ctxT_pool:
            ctxT = ctxT_pool.tile([128, 8, N], F32R)

            with tc.tile_pool(name="xTp", bufs=1) as xT_pool:
                xT = xT_pool.tile([128, 8, NQ], F32R)

                # ---- Phase A: transpose context and x-slice to feature-major
                with (
                    tc.tile_pool(name="nat", bufs=3) as nat_pool,
                    tc.tile_pool(name="pst", bufs=4, space="PSUM") as pst,
                ):
                    for i in range(N // 128):
                        cnat = nat_pool.tile([128, D], F32, tag="nat")
                        nc.sync.dma_start(out=cnat, in_=ctx.ap()[i * 128:(i + 1) * 128, :])
                        for j in range(8):
                            pt = pst.tile([128, 128], F32, tag="t")
                            nc.tensor.transpose(pt, cnat[:, j * 128:(j + 1) * 128], ident)
                            nc.vector.tensor_copy(
                                out=ctxT[:, j, i * 128:(i + 1) * 128], in_=pt)
                    for i in range(NQ // 128):
                        xnat = nat_pool.tile([128, D], F32, tag="nat")
                        nc.sync.dma_start(out=xnat, in_=xs.ap()[i * 128:(i + 1) * 128, :])
                        for j in range(8):
                            pt = pst.tile([128, 128], F32, tag="t")
                            nc.tensor.transpose(pt, xnat[:, j * 128:(j + 1) * 128], ident)
                            nc.vector.tensor_copy(
                                out=xT[:, j, i * 128:(i + 1) * 128], in_=pt)

                # ---- Phase B: Q^T = Wq^T x^T + bq
                with (
                    tc.tile_pool(name="wq", bufs=9) as wq_pool,
                    tc.tile_pool(name="psq", bufs=2, space="PSUM") as psq,
                ):
                    wq_t = []
                    for dk in range(8):
                        w = wq_pool.tile([128, D], F32R, tag="wq")
                        nc.sync.dma_start(out=w, in_=Wq.ap()[dk * 128:(dk + 1) * 128, :].bitcast(F32R))
                        wq_t.append(w)
                    for fm in range(8):
                        pq = psq.tile([128, NQ], F32, tag="q")
                        for dk in range(8):
                            nc.tensor.matmul(
                                pq, wq_t[dk][:, fm * 128:(fm + 1) * 128],
                                xT[:, dk, :], start=(dk == 0), stop=(dk == 7),
                            )
                        nc.vector.tensor_scalar(
                            out=qT[:, fm, :], in0=pq, scalar1=bq_c[:, fm:fm + 1],
                            scalar2=None, op0=mybir.AluOpType.add,
                        )

            # ---- Phase C: per head-group projections + attention
            for g in range(NG):
                with (
                    tc.tile_pool(name="kv", bufs=1) as kv_pool,
                    tc.tile_pool(name="wg", bufs=8) as wg_pool,
                ):
                    kT = kv_pool.tile([128, 2, N], F32R, tag="kT")
                    vg = kv_pool.tile([128, 16, 4, 65], F32R, tag="vg")
                    nc.vector.tensor_copy(out=vg[:, :, :, 64:65], in_=vones)

                    wk_t, wv_t = [], []
                    for dk in range(8):
                        w = wg_pool.tile([128, GF], F32R, tag="wk")
                        nc.sync.dma_start(
                            out=w, in_=Wk.ap()[dk * 128:(dk + 1) * 128, g * GF:(g + 1) * GF].bitcast(F32R))
                        wk_t.append(w)
                        w = wg_pool.tile([128, GF], F32R, tag="wv")
                        nc.sync.dma_start(
                            out=w, in_=Wv.ap()[dk * 128:(dk + 1) * 128, g * GF:(g + 1) * GF].bitcast(F32R))
                        wv_t.append(w)

                    with tc.tile_pool(name="psk", bufs=4, space="PSUM") as psk:
                        for gj in range(2):
                            pk = [
                                psk.tile([128, 512], F32, name=f"pk{tn}", tag="k")
                                for tn in range(4)
                            ]
                            for dk in range(8):
                                for tn in range(4):
                                    nc.tensor.matmul(
                                        pk[tn],
                                        wk_t[dk][:, gj * 128:(gj + 1) * 128],
                                        ctxT[:, dk, tn * 512:(tn + 1) * 512],
                                        start=(dk == 0), stop=(dk == 7),
                                    )
                            for tn in range(4):
                                nc.vector.tensor_scalar(
                                    out=kT[:, gj, tn * 512:(tn + 1) * 512], in0=pk[tn],
                                    scalar1=bk_c[:, 2 * g + gj:2 * g + gj + 1],
                                    scalar2=None, op0=mybir.AluOpType.add,
                                )

                    with tc.tile_pool(name="psv", bufs=3, space="PSUM") as psv:
                        bvg = bv_b[:, g * GF:(g + 1) * GF].rearrange("p (h c) -> p h c", h=4)
                        for kt in range(16):
                            pv = psv.tile([128, GF], F32, tag="v")
                            for dk in range(8):
                                nc.tensor.matmul(
                                    pv, ctxT[:, dk, kt * 128:(kt + 1) * 128], wv_t[dk],
                                    start=(dk == 0), stop=(dk == 7),
                                )
                            nc.vector.tensor_add(
                                out=vg[:, kt, :, 0:64],
                                in0=pv.rearrange("p (h c) -> p h c", h=4), in1=bvg,
                            )

                    # attention: two packed head pairs
                    with (
                        tc.tile_pool(name="pp", bufs=4) as pp,
                        tc.tile_pool(name="rp", bufs=2) as rp,
                        tc.tile_pool(name="pss", bufs=3, space="PSUM") as pss,
                        tc.tile_pool(name="pso", bufs=2, space="PSUM") as pso,
                        tc.tile_pool(name="psb", bufs=2, space="PSUM") as psb,
                    ):
                        for pi in range(2):
                            la, lb = 2 * pi, 2 * pi + 1
                            jq = 2 * g + pi
                            oA = pso.tile([128, NQ], F32, tag="o")
                            oB = pso.tile([128, NQ], F32, tag="o")
                            for kt in range(16):
                                sA = pss.tile([128, NQ], F32, tag="s")
                                sB = pss.tile([128, NQ], F32, tag="s")
                                ks = kT[:, pi, kt * 128:(kt + 1) * 128]
                                nc.tensor.matmul(
                                    sA, ks[0:64], qT[0:64, jq, :],
                                    start=True, stop=True, tile_position=(0, 0),
                                )
                                nc.tensor.matmul(
                                    sB, ks[64:128], qT[64:128, jq, :],
                                    start=True, stop=True, tile_position=(64, 0),
                                )
                                pA = pp.tile([128, NQ], F32R, tag="p")
                                pB = pp.tile([128, NQ], F32R, tag="p")
                                nc.scalar.activation(out=pA, in_=sA, func=AF.Exp, scale=SCALE)
                                nc.scalar.activation(out=pB, in_=sB, func=AF.Exp, scale=SCALE)
                                nc.tensor.matmul(
                                    oA[0:65, :], vg[:, kt, la, :], pA,
                                    start=(kt == 0), stop=(kt == 15),
                                )
                                nc.tensor.matmul(
                                    oB[0:65, :], vg[:, kt, lb, :], pB,
                                    start=(kt == 0), stop=(kt == 15),
                                )
                            rA = rp.tile([1, NQ], F32R, tag="rc")
                            rB = rp.tile([1, NQ], F32R, tag="rc")
                            nc.vector.reciprocal(out=rA, in_=oA[64:65, :])
                            nc.vector.reciprocal(out=rB, in_=oB[64:65, :])
                            bA = psb.tile([128, NQ], F32, tag="b")
                            bB = psb.tile([128, NQ], F32, tag="b")
                            nc.tensor.matmul(bA[0:64, :], ones64, rA, start=True, stop=True)
                            nc.tensor.matmul(bB[0:64, :], ones64, rB, start=True, stop=True)
                            stA = rp.tile([64, NQ], F32, tag="st")
                            stB = rp.tile([64, NQ], F32, tag="st")
                            nc.scalar.copy(out=stA, in_=oA[0:64, :])
                            nc.scalar.copy(out=stB, in_=oB[0:64, :])
                            nc.vector.tensor_mul(
                                out=attnT[0:64, jq, :], in0=stA, in1=bA[0:64, :])
                            nc.vector.tensor_mul(
                                out=attnT[64:128, jq, :], in0=stB, in1=bB[0:64, :])

        # ---- Phase D: output projection + residual + LayerNorm
        with (
            tc.tile_pool(name="wo", bufs=9) as wo_pool,
            tc.tile_pool(name="yb", bufs=2) as y_pool,
            tc.tile_pool(name="ln", bufs=4) as ln_pool,
            tc.tile_pool(name="psy", bufs=4, space="PSUM") as psy,
        ):
            wo_t = []
            for fk in range(8):
                w = wo_pool.tile([128, D], F32R, tag="wo")
                nc.sync.dma_start(out=w, in_=Wo.ap()[fk * 128:(fk + 1) * 128, :].bitcast(F32R))
                wo_t.append(w)
            yr = y.ap().rearrange("(i p) d -> p i d", p=128)
            for qm in range(4):
                ysb = y_pool.tile([128, D], F32, tag="y")
                for dn in range(2):
                    py = psy.tile([128, 512], F32, tag="y")
                    for fk in range(8):
                        nc.tensor.matmul(
                            py, attnT[:, fk, qm * 128:(qm + 1) * 128],
                            wo_t[fk][:, dn * 512:(dn + 1) * 512],
                            start=(fk == 0), stop=(fk == 7),
                        )
                    nc.vector.tensor_add(
                        out=ysb[:, dn * 512:(dn + 1) * 512], in0=py,
                        in1=xbo[:, qm, dn * 512:(dn + 1) * 512],
                    )
                st = ln_pool.tile([128, 2, 6], F32, tag="st")
                for s2 in range(2):
                    nc.vector.bn_stats(out=st[:, s2, :], in_=ysb[:, s2 * 512:(s2 + 1) * 512])
                mv = ln_pool.tile([128, 2], F32, tag="mv")
                nc.vector.bn_aggr(out=mv, in_=st)
                nc.scalar.activation(
                    out=mv[:, 1:2], in_=mv[:, 1:2], func=AF.Sqrt, bias=eps_t, scale=1.0)
                nc.vector.reciprocal(out=mv[:, 1:2], in_=mv[:, 1:2])
                nc.vector.tensor_scalar(
                    out=ysb, in0=ysb, scalar1=mv[:, 0:1], scalar2=mv[:, 1:2],
                    op0=mybir.AluOpType.subtract, op1=mybir.AluOpType.mult,
                )
                nc.vector.tensor_mul(out=ysb, in0=ysb, in1=gamma_b)
                nc.vector.tensor_add(out=ysb, in0=ysb, in1=beta_b)
                nc.sync.dma_start(out=yr[:, qm, :], in_=ysb)

    return nc


def build():
    if "nc" not in _CACHE:
        nc = bacc.Bacc(trn_type="TRN2", target_bir_lowering=False, debug=False)
        _emit(nc)
        nc.compile()
        _CACHE["nc"] = nc
    return _CACHE["nc"]


def make_in_maps(x, context, Wq, bq, Wk, bk, Wv, bv, Wo, bo, gamma, beta):
    f32 = lambda a: np.ascontiguousarray(np.asarray(a, dtype=np.float32))
    shared = {
        "Wq": f32(Wq), "Wk": f32(Wk), "Wv": f32(Wv), "Wo": f32(Wo),
        "bq": f32(bq), "bk": f32(bk), "bv": f32(bv), "bo": f32(bo),
        "gamma": f32(gamma), "beta": f32(beta),
    }
    x = f32(x)
    context = f32(context)
    in_maps = []
    for c in range(8):
        b, qi = c // 4, c % 4
        m = dict(shared)
        m["xs"] = np.ascontiguousarray(x[b, qi * NQ:(qi + 1) * NQ, :])
        m["ctx"] = context[b]
        in_maps.append(m)
    return in_maps


def gather(results):
    y = np.empty((B, N, D), np.float32)
    for c in range(8):
        b, qi = c // 4, c % 4
        y[b, qi * NQ:(qi + 1) * NQ, :] = results[c]["y"]
    return y


def kernel(**inputs):
    from concourse import bass_utils

    nc = build()
    in_maps = make_in_maps(**inputs)
    res = bass_utils.run_bass_kernel_spmd(nc, in_maps, core_ids=list(range(8)))
    return gather(res.results)



# revision 17
# speedup vs baseline: 1.4433x; 1.4433x over previous
"""CrossAttention + residual + LayerNorm on 8 Trainium2 NeuronCores.

Reference computation (per batch b):
    q = x @ Wq + bq ; k = ctx @ Wk + bk ; v = ctx @ Wv + bv      (16 heads of 64)
    attn = softmax(q k^T / 8) ; out = attn @ v
    y = LayerNorm(out @ Wo + bo + x) * gamma + beta

Sharding: core c -> batch b = c//4, query rows [512*(c%4), 512*(c%4+1)).
Each core recomputes K/V projections for its batch.

Perf design (v2): the TRN2 PE clock is HAM-gated — it runs at 1.2 GHz unless
the tensor engine is continuously busy (4096-cycle activity windows), at
2.4 GHz when it is. The v1 kernel ran each head-group's attention as a
dependent S->exp->O chain with micro-bubbles, so the PE spent ~450us of a
670us run throttled at half clock. v2 keeps the PE stream dense:
  * attention is software-pipelined: scores S(kt+2) and exp(kt+1) are issued
    ahead of the attn@V consumption O(kt);
  * head-group g's attention is interleaved, instruction by instruction, with
    head-group g+1's K/V projection matmuls (independent work that fills all
    PE gaps), and with the DMA+cast staging of group g+2's weights;
  * exp is issued as one [128,1024] ACT instruction per (pair, k-tile)
    spanning two PSUM banks, halving ACT instruction overhead;
  * softmax normalization divides by a PE-broadcast denominator row
    (matmul with a ones column) on DVE, skipping the slow 1-partition
    reciprocal;
  * ctx^T / Q^T / K^T / V / P are bf16 (matmul accumulation stays fp32 in
    PSUM), halving SBUF footprint and transpose cost.

Layouts on core (SBUF):
    ctxT  [128, 8, 2048] bf16  context^T  (feature f = 128*j + p)
    xT    [128, 8, 512]  bf16  x-slice^T
    qT    [128, 8, 512]  bf16  Q^T   head h at rows 64*(h%2) of block h//2
    kT_g  [128, 2, 2048] bf16  K^T for the 4-head group g
    v_g   [128, 16, 4, 66] bf16  V natural per k-tile, head-in-group; ones at
                                 [..., 64] (softmax denominator via matmul)
    attnT [128, 8, 512]  f32r  normalized attention output^T

Attention per head pair (A at PE rows 0-63, B at 64-127):
    S^T[k_tile, q] = (K^T tile).T @ Q^T        -> PSUM [128, 2, 512]
    P^T = exp(S^T / 8)                         -> SBUF bf16 (one ACT op)
    O^T[65, q] += (V_aug tile).T @ P^T         -> row 64 = sum_k P
    bcast = ones64^T @ denom_row (PE) ; attnT = O^T[0:64] / bcast (DVE)
"""

from contextlib import ExitStack

import numpy as np

import concourse.bacc as bacc
import concourse.bass as bass
import concourse.tile as tile
from concourse import mybir
from concourse.masks import make_identity

F32 = mybir.dt.float32
F32R = mybir.dt.float32r
BF16 = mybir.dt.bfloat16
AF = mybir.ActivationFunctionType
ADD = mybir.AluOpType.add
SUB = mybir.AluOpType.subtract
MUL = mybir.AluOpType.mult
DIV = mybir.AluOpType.divide

B = 2
N = 2048          # context length
D = 1024          # model dim
H = 16            # heads
HD = 64           # head dim
NQ = 512          # query rows per core
SCALE = HD ** -0.5
EPS = 1e-5
NG = 4            # head groups
GF = D // NG      # features per group (256)

_CACHE = {}

_SENT = object()


def _drain(gen, n):
    """Pull up to n steps from generator; no-op when exhausted."""
    for _ in range(n):
        if next(gen, _SENT) is _SENT:
            return False
    return True


def _drain_all(gen):
    for _ in gen:
        pass


def _empty_gen():
    return iter(())


def _emit(nc):
    with nc.allow_low_precision(reason="bf16 matmul operands; fp32 PSUM accum"):
        _emit_body(nc)


def _emit_body(nc):
    xs = nc.dram_tensor("xs", [NQ, D], F32, kind="ExternalInput")
    ctx = nc.dram_tensor("ctx", [N, D], F32, kind="ExternalInput")
    Wq = nc.dram_tensor("Wq", [D, D], F32, kind="ExternalInput")
    Wk = nc.dram_tensor("Wk", [D, D], F32, kind="ExternalInput")
    Wv = nc.dram_tensor("Wv", [D, D], F32, kind="ExternalInput")
    Wo = nc.dram_tensor("Wo", [D, D], F32, kind="ExternalInput")
    bq = nc.dram_tensor("bq", [D], F32, kind="ExternalInput")
    bk = nc.dram_tensor("bk", [D], F32, kind="ExternalInput")
    bv = nc.dram_tensor("bv", [D], F32, kind="ExternalInput")
    bo = nc.dram_tensor("bo", [D], F32, kind="ExternalInput")
    gamma = nc.dram_tensor("gamma", [D], F32, kind="ExternalInput")
    beta = nc.dram_tensor("beta", [D], F32, kind="ExternalInput")
    y = nc.dram_tensor("y", [NQ, D], F32, kind="ExternalOutput")

    def bcast_row(dram_vec):
        # [D] -> [128, D] DMA broadcast (partition step 0)
        a = dram_vec.ap()
        return bass.AP(tensor=a.tensor, offset=0, ap=[[0, 128]] + a.ap)

    def col_view(dram_vec):
        # [D] -> [128, 8] with [p, j] = vec[128*j + p]
        return dram_vec.ap().rearrange("(j p) -> p j", p=128)

    with ExitStack() as es:
        tc = es.enter_context(tile.TileContext(nc))
        pool = lambda **kw: es.enter_context(tc.tile_pool(**kw))
        const = pool(name="const", bufs=1)
        resid = pool(name="resid", bufs=1)
        qT_pool = pool(name="qTp", bufs=1)
        attnT_pool = pool(name="attnTp", bufs=1)
        ctxT_pool = pool(name="ctxTp", bufs=1)
        kv_pool = pool(name="kvp", bufs=2)
        wkv_pool = pool(name="wkvp", bufs=2)
        wst_pool = pool(name="wstp", bufs=2)
        pp_pool = pool(name="ppp", bufs=3)
        rp_pool = pool(name="rpp", bufs=2)
        psk = pool(name="psk", bufs=1, space="PSUM")
        psv = pool(name="psv", bufs=1, space="PSUM")
        ident = const.tile([128, 128], F32)
        make_identity(nc, ident)
        # f32r copy (rounded-on-write) — the BIR verifier rejects bitcast
        # views of compute-written fp32 tensors as f32r matmul operands.
        identR = const.tile([128, 128], F32R)
        nc.vector.tensor_copy(out=identR, in_=ident)
        # ones row lives at partition 64 to match the denominator row of the
        # attention accumulator (O^T row 64) it multiplies against.
        ones64f = const.tile([65, 64], F32)
        nc.vector.memset(ones64f, 1.0)
        ones64 = const.tile([65, 64], F32R)
        nc.vector.tensor_copy(out=ones64, in_=ones64f)
        ones_row = ones64[64:65, :]
        vones = const.tile([128, 16, 4, 1], BF16)
        nc.vector.memset(vones, 1.0)
        eps_t = const.tile([128, 1], F32)
        nc.vector.memset(eps_t, EPS)
        bq_c = const.tile([128, 8], F32)
        nc.sync.dma_start(out=bq_c, in_=col_view(bq))
        bk_c = const.tile([128, 8], F32)
        nc.sync.dma_start(out=bk_c, in_=col_view(bk))
        bv_b = const.tile([128, D], F32)
        nc.sync.dma_start(out=bv_b, in_=bcast_row(bv))
        bo_b = const.tile([128, D], F32)
        nc.sync.dma_start(out=bo_b, in_=bcast_row(bo))
        gamma_b = const.tile([128, D], F32)
        nc.sync.dma_start(out=gamma_b, in_=bcast_row(gamma))
        beta_b = const.tile([128, D], F32)
        nc.sync.dma_start(out=beta_b, in_=bcast_row(beta))

        qT = qT_pool.tile([128, 8, NQ], BF16)
        attnT = attnT_pool.tile([128, 8, NQ], F32R)
        ctxT = ctxT_pool.tile([128, 8, N], BF16)
        xbo = resid.tile([128, 4, D], F32)   # x-slice (+bo later), t = 128*i + p

        # -- staged weight tiles (bf16), rotated 2-deep across groups
        wk_tiles = {g: [None] * 8 for g in range(NG)}
        wv_tiles = {g: [None] * 8 for g in range(NG)}

        def stage_steps(g):
            """DMA fp32 slices of Wk/Wv for group g and cast to bf16."""
            for dk in range(8):
                for W, dst, nm in ((Wk, wk_tiles, "k"), (Wv, wv_tiles, "v")):
                    st = wst_pool.tile([128, GF], F32, tag="wst", name=f"wst{nm}{g}{dk}")
                    nc.sync.dma_start(
                        out=st, in_=W.ap()[dk * 128:(dk + 1) * 128, g * GF:(g + 1) * GF])
                    t = wkv_pool.tile([128, GF], BF16, tag=f"w{nm}{dk}", name=f"w{nm}{g}{dk}")
                    nc.vector.tensor_copy(out=t, in_=st)
                    dst[g][dk] = t
                    yield

        def kvproj_steps(g, kT, vg):
            """K^T and V projections for group g: 32 PE chunks of ~8 matmuls."""
            wkb, wvb = wk_tiles[g], wv_tiles[g]
            nc.vector.tensor_copy(out=vg[:, :, :, 64:65], in_=vones)
            bvg = bv_b[:, g * GF:(g + 1) * GF].rearrange("p (h c) -> p h c", h=4)
            for gj in range(2):
                for tn in range(8):
                    if tn % 2 == 0:
                        pk = psk.tile([128, 2, GF], F32, tag="k", name=f"pk{g}{gj}{tn}")
                    sl = pk[:, tn % 2, :]
                    for dk in range(8):
                        nc.tensor.matmul(
                            sl, wkb[dk][:, gj * 128:(gj + 1) * 128],
                            ctxT[:, dk, tn * GF:(tn + 1) * GF],
                            start=(dk == 0), stop=(dk == 7),
                        )
                    yield
                    nc.vector.tensor_scalar(
                        out=kT[:, gj, tn * GF:(tn + 1) * GF], in0=sl,
                        scalar1=bk_c[:, 2 * g + gj:2 * g + gj + 1],
                        scalar2=None, op0=ADD,
                    )
            for kt2 in range(8):
                pv = psv.tile([128, 2, GF], F32, tag="v", name=f"pv{g}{kt2}")
                for half in range(2):
                    kt = 2 * kt2 + half
                    for dk in range(8):
                        nc.tensor.matmul(
                            pv[:, half, :], ctxT[:, dk, kt * 128:(kt + 1) * 128],
                            wvb[dk], start=(dk == 0), stop=(dk == 7),
                        )
                    yield
                    nc.vector.tensor_add(
                        out=vg[:, kt, :, 0:64],
                        in0=pv[:, half, :].rearrange("p (h c) -> p h c", h=4),
                        in1=bvg,
                    )

        def alloc_kv(g):
            kT = kv_pool.tile([128, 2, N], BF16, tag="kT", name=f"kT{g}")
            vg = kv_pool.tile([128, 16, 4, 66], BF16, tag="vg", name=f"vg{g}")
            return kT, vg

        def transpose_128x1024(trp, src_tile, dst, copy_pick, fast):
            """Transpose a [128, 1024] fp32 natural tile into feature-major
            bf16 dst [128, 8, 128]; copy-outs alternate DVE/ACT per copy_pick.
            fast=True uses the f32r transpose path (1.5 vs 2 cycles/row) —
            only valid when src_tile is DMA-written (verifier constraint)."""
            for half in range(2):
                if fast:
                    pt = trp.tile([128, 512], F32R, tag="tr", name="pt")
                else:
                    pt = trp.tile([128, 512], F32, tag="trx", name="pt")
                for jj in range(4):
                    j = 4 * half + jj
                    src_j = src_tile[:, j * 128:(j + 1) * 128]
                    if fast:
                        nc.tensor.transpose(
                            pt[:, jj * 128:(jj + 1) * 128], src_j, identR)
                    else:
                        nc.tensor.transpose(
                            pt[:, jj * 128:(jj + 1) * 128], src_j, ident)
                src = pt.rearrange("p (j c) -> p j c", j=4)
                d = dst[:, 4 * half:4 * half + 4, :]
                if copy_pick(half):
                    nc.vector.tensor_copy(out=d, in_=src)
                else:
                    nc.scalar.copy(out=d, in_=src)

        def prologue(xT_pool, wq_pool, nat_pool, trp, psq):
            xT = xT_pool.tile([128, 8, NQ], BF16)

            # ---- ctx -> ctxT (PE transpose, copy-out via DVE/ACT alternating)
            for i in range(N // 128):
                cnat = nat_pool.tile([128, D], F32R, tag="nat")
                nc.sync.dma_start(
                    out=cnat,
                    in_=ctx.ap()[i * 128:(i + 1) * 128, :].bitcast(F32R))
                transpose_128x1024(
                    trp, cnat, ctxT[:, :, i * 128:(i + 1) * 128],
                    lambda half, i=i: (2 * i + half) % 2 == 0, fast=True)

            # group-0 weights (DMAs queue behind ctx)
            _drain_all(stage_steps(0))

            # x natural (residual base + transpose source)
            nc.sync.dma_start(
                out=xbo, in_=xs.ap().rearrange("(i p) d -> p i d", p=128))

            # Wq: DMA fp32 staging (256-col slices) -> bf16
            wqb = []
            for dk in range(8):
                t = wq_pool.tile([128, D], BF16, tag=f"wq{dk}", name=f"wq{dk}")
                for qs in range(4):
                    st = wst_pool.tile(
                        [128, GF], F32, tag="wst", name=f"wqst{dk}{qs}")
                    nc.sync.dma_start(
                        out=st,
                        in_=Wq.ap()[dk * 128:(dk + 1) * 128, qs * GF:(qs + 1) * GF])
                    nc.vector.tensor_copy(
                        out=t[:, qs * GF:(qs + 1) * GF], in_=st)
                wqb.append(t)

            # K/V projections for group 0 (PE-dense; runs while x/Wq stream in)
            kv0 = alloc_kv(0)
            _drain_all(kvproj_steps(0, *kv0))

            # x -> xT
            for i in range(4):
                transpose_128x1024(
                    trp, xbo[:, i, :], xT[:, :, i * 128:(i + 1) * 128],
                    lambda half: half == 0, fast=False)
            # residual base: x + bo (after transposes read x)
            for i in range(4):
                nc.vector.tensor_add(out=xbo[:, i, :], in0=xbo[:, i, :], in1=bo_b)

            # Q^T = (Wq^T x^T) + bq, interleaved with group-1 weight staging
            stage1 = stage_steps(1)
            for fm in range(8):
                pq = psq.tile([128, NQ], F32, tag="q", name="pq")
                for dk in range(8):
                    nc.tensor.matmul(
                        pq, wqb[dk][:, fm * 128:(fm + 1) * 128],
                        xT[:, dk, :], start=(dk == 0), stop=(dk == 7),
                    )
                _drain(stage1, 2)
                nc.vector.tensor_scalar(
                    out=qT[:, fm, :], in0=pq, scalar1=bq_c[:, fm:fm + 1],
                    scalar2=None, op0=ADD,
                )
            _drain_all(stage1)
            return kv0

        # ================= Prologue =================
        # DMA queue order: ctx (needed first), Wk/Wv group 0, x, Wq.
        with ExitStack() as pes:
            ppool = lambda **kw: pes.enter_context(tc.tile_pool(**kw))
            kv0 = prologue(
                ppool(name="xTp", bufs=1), ppool(name="wqp", bufs=1),
                ppool(name="natp", bufs=3),
                ppool(name="trp", bufs=2, space="PSUM"),
                ppool(name="psq", bufs=2, space="PSUM"))

        # ================= Attention groups =================
        kv_cur = kv0
        wo_pool = pool(name="wop", bufs=1)
        with tc.tile_pool(name="pss", bufs=2, space="PSUM") as pss, \
             tc.tile_pool(name="pso", bufs=1, space="PSUM") as pso:
            for g in range(NG):
                kT, vg = kv_cur
                if g + 1 < NG:
                    kv_nxt = alloc_kv(g + 1)
                    proj_gen = kvproj_steps(g + 1, *kv_nxt)
                else:
                    kv_nxt = None
                    proj_gen = _empty_gen()
                stage_gen = stage_steps(g + 2) if g + 2 < NG else _empty_gen()
                if g == 2:
                    # prefetch Wo during group 2's window (f32r: plain bitcast)
                    wob = []
                    for fk in range(8):
                        w = wo_pool.tile([128, D], F32R, tag=f"wo{fk}", name=f"wo{fk}")
                        nc.sync.dma_start(
                            out=w,
                            in_=Wo.ap()[fk * 128:(fk + 1) * 128, :].bitcast(F32R))
                        wob.append(w)

                for pi in range(2):
                    la, lb = 2 * pi, 2 * pi + 1
                    jq = 2 * g + pi
                    oAB = pso.tile([128, 2, NQ], F32, tag="o", name=f"o{g}{pi}")
                    sab = [None, None]
                    pab = [None, None]

                    def emit_S(kt, g=g, pi=pi, jq=jq, kT=kT, sab=sab):
                        s = pss.tile([128, 2, NQ], F32, tag="s", name=f"s{g}{pi}{kt}")
                        ks = kT[:, pi, kt * 128:(kt + 1) * 128]
                        nc.tensor.matmul(
                            s[:, 0, :], ks[0:64], qT[0:64, jq, :],
                            start=True, stop=True, tile_position=(0, 0),
                        )
                        nc.tensor.matmul(
                            s[:, 1, :], ks[64:128], qT[64:128, jq, :],
                            start=True, stop=True, tile_position=(64, 0),
                        )
                        sab[kt % 2] = s

                    def emit_exp(kt, g=g, pi=pi, sab=sab, pab=pab):
                        p = pp_pool.tile(
                            [128, 2, NQ], BF16, tag="p", name=f"p{g}{pi}{kt}")
                        nc.scalar.activation(
                            out=p, in_=sab[kt % 2], func=AF.Exp, scale=SCALE)
                        pab[kt % 2] = p

                    emit_S(0)
                    emit_exp(0)
                    emit_S(1)
                    for kt in range(16):
                        _drain(proj_gen, 1)
                        _drain(stage_gen, 1)
                        p = pab[kt % 2]
                        nc.tensor.matmul(
                            oAB[0:65, 0, :], vg[:, kt, la, 0:65], p[:, 0, :],
                            start=(kt == 0), stop=(kt == 15),
                        )
                        nc.tensor.matmul(
                            oAB[0:65, 1, :], vg[:, kt, lb, 0:65], p[:, 1, :],
                            start=(kt == 0), stop=(kt == 15),
                        )
                        if kt + 2 < 16:
                            emit_S(kt + 2)
                        if kt + 1 < 16:
                            emit_exp(kt + 1)
                    # tail: denominator broadcast + divide
                    stAB = rp_pool.tile([65, 2, NQ], F32R, tag="st", name=f"st{g}{pi}")
                    nc.scalar.copy(out=stAB, in_=oAB[0:65, :, :])
                    rAB = rp_pool.tile([1, 2, NQ], F32R, tag="r", name=f"r{g}{pi}")
                    nc.vector.reciprocal(out=rAB[:, 0, :], in_=stAB[64:65, 0, :])
                    nc.vector.reciprocal(out=rAB[:, 1, :], in_=stAB[64:65, 1, :])
                    bAB = pss.tile([128, 2, NQ], F32, tag="s", name=f"b{g}{pi}")
                    nc.tensor.matmul(
                        bAB[0:64, 0, :], ones64[0:1, :], rAB[:, 0, :],
                        start=True, stop=True)
                    nc.tensor.matmul(
                        bAB[0:64, 1, :], ones64[0:1, :], rAB[:, 1, :],
                        start=True, stop=True)
                    nc.vector.tensor_mul(
                        out=attnT[0:64, jq, :], in0=stAB[0:64, 0, :],
                        in1=bAB[0:64, 0, :])
                    nc.vector.tensor_mul(
                        out=attnT[64:128, jq, :], in0=stAB[0:64, 1, :],
                        in1=bAB[0:64, 1, :])
                _drain_all(proj_gen)
                _drain_all(stage_gen)
                kv_cur = kv_nxt

        # ================= Output projection + residual + LayerNorm =========
        with tc.tile_pool(name="yb", bufs=2) as y_pool, \
             tc.tile_pool(name="ln", bufs=4) as ln_pool, \
             tc.tile_pool(name="psy", bufs=2, space="PSUM") as psy:
            yr = y.ap().rearrange("(i p) d -> p i d", p=128)
            for qm in range(4):
                ysb = y_pool.tile([128, D], F32, tag="y")
                for dn in range(2):
                    py = psy.tile([128, 512], F32, tag="y")
                    for fk in range(8):
                        nc.tensor.matmul(
                            py, attnT[:, fk, qm * 128:(qm + 1) * 128],
                            wob[fk][:, dn * 512:(dn + 1) * 512],
                            start=(fk == 0), stop=(fk == 7),
                        )
                    nc.vector.tensor_add(
                        out=ysb[:, dn * 512:(dn + 1) * 512], in0=py,
                        in1=xbo[:, qm, dn * 512:(dn + 1) * 512],
                    )
                st = ln_pool.tile([128, 2, 6], F32, tag="st")
                for s2 in range(2):
                    nc.vector.bn_stats(out=st[:, s2, :], in_=ysb[:, s2 * 512:(s2 + 1) * 512])
                mv = ln_pool.tile([128, 2], F32, tag="mv")
                nc.vector.bn_aggr(out=mv, in_=st)
                nc.scalar.activation(
                    out=mv[:, 1:2], in_=mv[:, 1:2], func=AF.Sqrt, bias=eps_t, scale=1.0)
                nc.vector.reciprocal(out=mv[:, 1:2], in_=mv[:, 1:2])
                nc.vector.tensor_scalar(
                    out=ysb, in0=ysb, scalar1=mv[:, 0:1], scalar2=mv[:, 1:2],
                    op0=SUB, op1=MUL,
                )
                nc.vector.tensor_mul(out=ysb, in0=ysb, in1=gamma_b)
                nc.vector.tensor_add(out=ysb, in0=ysb, in1=beta_b)
                nc.sync.dma_start(out=yr[:, qm, :], in_=ysb)

    return nc


def build():
    if "nc" not in _CACHE:
        nc = bacc.Bacc(trn_type="TRN2", target_bir_lowering=False, debug=False)
        _emit(nc)
        nc.compile()
        _CACHE["nc"] = nc
    return _CACHE["nc"]


def make_in_maps(x, context, Wq, bq, Wk, bk, Wv, bv, Wo, bo, gamma, beta):
    f32 = lambda a: np.ascontiguousarray(np.asarray(a, dtype=np.float32))
    shared = {
        "Wq": f32(Wq), "Wk": f32(Wk), "Wv": f32(Wv), "Wo": f32(Wo),
        "bq": f32(bq), "bk": f32(bk), "bv": f32(bv), "bo": f32(bo),
        "gamma": f32(gamma), "beta": f32(beta),
    }
    x = f32(x)
    context = f32(context)
    in_maps = []
    for c in range(8):
        b, qi = c // 4, c % 4
        m = dict(shared)
        m["xs"] = np.ascontiguousarray(x[b, qi * NQ:(qi + 1) * NQ, :])
        m["ctx"] = context[b]
        in_maps.append(m)
    return in_maps


def gather(results):
    y = np.empty((B, N, D), np.float32)
    for c in range(8):
        b, qi = c // 4, c % 4
        y[b, qi * NQ:(qi + 1) * NQ, :] = results[c]["y"]
    return y


def kernel(**inputs):
    from concourse import bass_utils

    nc = build()
    in_maps = make_in_maps(**inputs)
    res = bass_utils.run_bass_kernel_spmd(nc, in_maps, core_ids=list(range(8)))
    return gather(res.results)
